# revision 25
# baseline (speedup 1.0000x reference)
"""Trainium2 Bass kernel for mixed-head attention (CIM attention).

Reference computation (per batch element b):
    qkv  = x @ w_qkv.T                                  [N, 3C]
    q,k,v split into H=4 heads of HD=128
    S_h  = (q_h @ k_h.T) * SCALE                        [N, N] per head
    S'_i = sum_h M[i,h] * S_h        (CIM head mix)
    A_i  = softmax(S'_i, axis=-1)
    O_i  = A_i @ v_i
    out  = concat_i(O_i) @ w_proj.T + b_proj

Distribution: data-parallel over B=8, one batch element per NeuronCore.
No collectives needed; host shards/gathers.

Single-core algorithm (all matmuls bf16 with fp32 PSUM accumulation):
  - Host ships x^T, w_qkv^T, w_proj^T pre-transposed, pre-cast to bf16 and
    pre-packed into flat "SBUF image" layouts, so every load is one
    contiguous DMA and the contraction dim is always on SBUF partitions.
    No device transposes anywhere.
  - Concurrent DMAs share HBM bandwidth round-robin per descriptor (not
    FIFO), so the ramp-critical loads (wq jb0 block, x^T ch0 halves, the
    remaining QK weights) are the only transfers in flight at the start;
    everything else (x^T ch1, wv, w_proj, bias) is issued from the scalar
    engine's program *between* the first epilogue copies, which delays
    those DIRECT2Ds until the QK phase is underway.
  - The PE p-state ramps 0.65 -> 1.2 -> 2.4 GHz with sustained activity
    (~2x slower matmuls for the first ~4-9us).  Eight dummy matmuls over
    memset data burn the ramp while the first input DMAs are in flight,
    so real chains start at high clock.
  - The CIM mix is folded into Q: Qhat_i[(h,d), n] = M[i,h]*SCALE*Q_h[d, n].
    Each Q projection tile is cast once PSUM->SBUF (ACT), then scaled into
    the 4 i-variants on DVE (bf16 fast mode, per-partition scalar).  The
    score matmul then contracts over all 512 (h,d) pairs:
    S'_i^T[m, n] = sum_{(h,d)} K[(h,d), m] * Qhat_i[(h,d), n].
  - Scores live in [m_part, n_free] ("S^T") layout so exp is elementwise and
    attn@v needs no transpose: O_i^T[d, n] = sum_m V[m, d] * expS_i^T[m, n].
  - Softmax normalization is deferred past attn@v.  The denominators come
    from a DVE add-tree that pre-reduces the 8 exp tiles to 1 (tile sums
    partial-sum the m axis), then one ones[128,128] stationary matmul whose
    M=128 output broadcasts the rowsum to all partitions for free;
    1/rowsum via reciprocal_approx_fast, applied to O^T with tensor_mul.
  - proj: out[n, c] = sum_{(i,d)} Onorm_i^T[(i,d), n] * w_proj^T[(i,d), c],
    emitted last so the scheduler backfills its matmuls into PE bubbles;
    b_proj is added during the PSUM->SBUF output copy from a
    host-pre-broadcast [128, C] bias tile.  Output is stored bf16 (halves
    the output DMA) and upcast on host.
"""

import os
import sys

for _p in ("/opt/trn_rl_repo",):
    if os.path.isdir(_p) and _p not in sys.path:
        sys.path.insert(0, _p)

import numpy as np
import ml_dtypes

import concourse.bass as bass
import concourse.tile as tile
from concourse import bacc, mybir
from concourse.bass_utils import run_bass_kernel_spmd

B, N, C, H = 8, 1024, 512, 4
HD = C // H          # 128
SCALE = HD ** -0.5
NCORES = 8
P = 128              # partitions
NCH = N // 512       # 512-wide free-dim chunks per N
NB = N // P          # 128-row blocks per N
CB = C // P          # 128-row blocks per C

BF16 = mybir.dt.bfloat16
FP32 = mybir.dt.float32
AF = mybir.ActivationFunctionType


def _mix_matrix_np(w_main: np.ndarray, w_rest: np.ndarray) -> np.ndarray:
    rows = np.repeat(np.arange(H), H - 1)
    cols = np.array([[j for j in range(H) if j != i] for i in range(H)]).ravel()
    M = np.zeros((H, H), dtype=np.float64)
    M[rows, cols] = w_rest.astype(np.float64).ravel()
    M += np.diag(w_main.astype(np.float64))
    return M


def build_graph():
    nc = bacc.Bacc(
        "TRN2",
        target_bir_lowering=False,
        debug=False,
        num_devices=NCORES,
    )

    # Priority-chunked input layouts.
    # wq0/wqr: w_qkv^T Q+K columns packed jb-major (jb = 8 output 128-col
    #   blocks: Q heads 0-3 then K heads 0-3), cb-minor: chunk jb holds the
    #   four [128,128] cb-blocks side by side.
    # xt0/xt1: x^T packed ch-major (ch = 512-wide n chunk), cb-minor: chunk
    #   ch holds four [128,512] cb-blocks side by side.
    wq0 = nc.dram_tensor("wq0", [P, CB * P], BF16, kind="ExternalInput").ap()
    wqr = nc.dram_tensor("wqr", [P, 7 * CB * P], BF16, kind="ExternalInput").ap()
    xt0 = nc.dram_tensor("xt0", [P, CB * 512], BF16, kind="ExternalInput").ap()
    xt1 = nc.dram_tensor("xt1", [P, CB * 512], BF16, kind="ExternalInput").ap()
    wv = nc.dram_tensor("wv", [P, CB * C], BF16, kind="ExternalInput").ap()
    wpTp = nc.dram_tensor("wpTp", [P, CB * C], BF16, kind="ExternalInput").ap()
    bprow = nc.dram_tensor("bprow", [P, C], BF16, kind="ExternalInput").ap()
    qscales = nc.dram_tensor("qscales", [P, H * H], FP32, kind="ExternalInput").ap()
    out = nc.dram_tensor("out", [N, C], BF16, kind="ExternalOutput").ap()

    with tile.TileContext(nc, pool_alloc_mode="queue") as tc:
        with (
            tc.tile_pool(name="const", bufs=1) as cpool,
            tc.tile_pool(name="wts", bufs=1) as wpool,
            tc.tile_pool(name="qkv", bufs=1) as qkvpool,
            tc.tile_pool(name="es", bufs=12) as espool,
            tc.tile_pool(name="onorm", bufs=1) as opool,
            tc.tile_pool(name="outsb", bufs=3) as outpool,
            tc.tile_pool(name="ps2", bufs=2, space="PSUM") as ps2pool,
            tc.tile_pool(name="psmm", bufs=4, space="PSUM") as psmm,
        ):
            # ---- priority-ordered input DMA (issuable engines: sync/SP,
            # scalar/ACT, gpsimd; ~0.7us serial per dma_start on a
            # sequencer, ~1.4us trigger->data latency).
            # Concurrent DMAs share bandwidth round-robin per descriptor
            # (NOT FIFO across transfers), so non-critical loads must not
            # be in flight while the critical Q/K feed streams.  Critical
            # now: wq0+xt0 halves, then wqr.  Everything else (xt1, wv,
            # wpp, bpr) is issued from the scalar engine INTERLEAVED with
            # the first epilogue copies, so those DMAs trigger only once
            # the QK phase is underway (program order on the sequencer
            # delays them past the copies' semaphore waits).
            warm = cpool.tile([P, 512], BF16, tag="warm")
            nc.gpsimd.memset(warm[:], 0.0)
            ones_m = cpool.tile([P, P], BF16, tag="ones_m")
            nc.gpsimd.memset(ones_m[:], 1.0)

            # xt0 in four per-cb chunks (completion-event granularity: the
            # first chain's cb-steps unblock as each 128 KB lands instead
            # of waiting a 256 KB half); wqr in three progressive chunks
            # matched to jb consumption order.
            xt0_sb = wpool.tile([P, CB * 512], BF16, tag="xt0", name="xt0")
            wq0_sb = wpool.tile([P, CB * P], BF16, tag="wq0", name="wq0")
            nc.sync.dma_start(xt0_sb[:, 0:512], xt0[:, 0:512])
            nc.scalar.dma_start(wq0_sb[:], wq0[:, :])
            nc.sync.dma_start(xt0_sb[:, 1024:1536], xt0[:, 1024:1536])
            nc.scalar.dma_start(xt0_sb[:, 512:1024], xt0[:, 512:1024])
            nc.scalar.dma_start(xt0_sb[:, 1536:2048], xt0[:, 1536:2048])

            wqr_sb = wpool.tile([P, 7 * CB * P], BF16, tag="wqr", name="wqr")
            nc.sync.dma_start(wqr_sb[:, 0:1024], wqr[:, 0:1024])
            nc.sync.dma_start(wqr_sb[:, 1024:2048], wqr[:, 1024:2048])
            nc.sync.dma_start(wqr_sb[:, 2048:3584], wqr[:, 2048:3584])
            qsc = cpool.tile([P, H * H], FP32, tag="qsc")
            nc.scalar.dma_start(qsc[:], qscales[:, :])

            # allocated here, loaded from inside the QKV loop (below)
            xt1_sb = wpool.tile([P, CB * 512], BF16, tag="xt1", name="xt1")
            wvp = wpool.tile([P, CB * C], BF16, tag="wvp", name="wvp")
            wpp = wpool.tile([P, CB * C], BF16, tag="wpp", name="wpp")
            bpr = cpool.tile([P, C], BF16, tag="bpr")

            def w_qk(jb, cb):
                if jb == 0:
                    return wq0_sb[:, cb * P:(cb + 1) * P]
                return wqr_sb[:, (jb - 1) * CB * P + cb * P:
                              (jb - 1) * CB * P + (cb + 1) * P]

            def xt(ch, cb):
                t = xt0_sb if ch == 0 else xt1_sb
                return t[:, cb * 512:(cb + 1) * 512]

            def xt_mb(cb, mb):
                # [128,128] m-block mb of cb-block cb (V projection lhsT)
                ch, j = divmod(mb, CB)
                t = xt0_sb if ch == 0 else xt1_sb
                return t[:, cb * 512 + j * P:cb * 512 + (j + 1) * P]

            wv_sb = [wvp[:, cb * C:(cb + 1) * C] for cb in range(CB)]
            wp_sb = [wpp[:, cb * C:(cb + 1) * C] for cb in range(CB)]

            # ---- QKV projections ----
            # qhat[i][h]: [128(d), N] bf16 ; kt[h]: [128(d), N] ; v[mb]: [128(m), C]
            qhat = [[qkvpool.tile([P, N], BF16, tag=f"qhat{i}_{h}",
                                  name=f"qhat{i}_{h}")
                     for h in range(H)] for i in range(H)]
            kt = [qkvpool.tile([P, N], BF16, tag=f"kt{h}", name=f"kt{h}")
                  for h in range(H)]
            v_sb = [qkvpool.tile([P, C], BF16, tag=f"v{mb}", name=f"v{mb}")
                    for mb in range(NB)]

            # PE p-state warmup: the tensor engine ramps 0.65 -> 1.2 -> 2.4
            # GHz with sustained activity (~2x slower matmuls for the first
            # ~9us of PE busy).  Burn that ramp on dummy matmuls over
            # memset data while the first input DMAs are still in flight,
            # so the real chains start at high clock.
            # 6 dummies end right as the first real operands land (~10.5us);
            # more would delay real work, fewer would leave the first
            # (data-stalled anyway) real chains at mid p-state
            ps_warm = psmm.tile([P, 512], FP32, tag="mm", name="warm_ps")
            for w in range(6):
                nc.tensor.matmul(ps_warm[:], ones_m[:], warm[:],
                                 start=True, stop=True)

            # Q and K: one [128, 512] chain per (ch, jb) so the whole ch=0
            # half runs off the first x^T chunk (xt1 lands while ch=0
            # computes); per-chain epilogue copy (ACT) + scaled qhat
            # variants (DVE).
            for ch in range(NCH):
                csl = slice(ch * 512, (ch + 1) * 512)
                for jb in range(2 * H):      # 0-3: Q heads, 4-7: K heads
                    ps = psmm.tile([P, 512], FP32, tag="mm",
                                   name=f"qk_ps{ch}_{jb}")
                    for cb in range(CB):
                        nc.tensor.matmul(
                            ps[:],
                            w_qk(jb, cb),
                            xt(ch, cb),
                            start=(cb == 0), stop=(cb == CB - 1),
                        )
                    if jb < H:
                        h = jb
                        # PSUM->SBUF cast on ACT, then 4 cheap bf16 scaled
                        # copies on DVE (CIM mix scales folded into qhat)
                        qb = qkvpool.tile([P, 512], BF16, tag="qb", bufs=8,
                                          name=f"qb{ch}_{h}")
                        nc.scalar.copy(qb[:], ps[:])
                        for i in range(H):
                            sc = qsc[:, i * H + h:i * H + h + 1]
                            nc.vector.tensor_scalar_mul(
                                qhat[i][h][:, csl], qb[:], sc)
                    else:
                        h = jb - H
                        nc.scalar.copy(kt[h][:, csl], ps[:])
                    if ch == 0:
                        # deferred non-critical loads: the DIRECT2D sits
                        # after this epilogue copy in scalar program order,
                        # keeping early HBM bandwidth for the QK feed
                        if jb == 0:
                            nc.scalar.dma_start(xt1_sb[:], xt1[:, :])
                        elif jb == 1:
                            nc.scalar.dma_start(wvp[:], wv[:, :])
                        elif jb == 2:
                            nc.scalar.dma_start(wpp[:], wpTp[:, :])
                        elif jb == 3:
                            nc.scalar.dma_start(bpr[:], bprow[:, :])

            # V: out[m_block, c] = sum_cb xT[cb][:, mblk].T @ wvT[cb]
            for mb in range(NB):
                ps = psmm.tile([P, 512], FP32, tag="mm", name=f"v_ps{mb}")
                for cb in range(CB):
                    nc.tensor.matmul(
                        ps[:],
                        xt_mb(cb, mb),
                        wv_sb[cb][:],
                        start=(cb == 0), stop=(cb == CB - 1),
                    )
                nc.vector.tensor_copy(v_sb[mb][:], ps[:])

            # ---- chunk-outer head loop: scores -> exp -> rowsum/attnv ----
            onorm = [opool.tile([P, N], BF16, tag=f"onorm{i}", name=f"onorm{i}")
                     for i in range(H)]

            for ch in range(NCH):
                nsl = slice(ch * 512, (ch + 1) * 512)
                for i in range(H):
                    es = [espool.tile([P, 512], BF16, tag="es",
                                      name=f"es{ch}_{i}_{mb}")
                          for mb in range(NB)]
                    ps_rso = ps2pool.tile([P, N], FP32, tag="mm2",
                                          name=f"rso{ch}_{i}")
                    ps_rs = ps_rso[:, 0:512]
                    ps_o = ps_rso[:, 512:1024]
                    # DVE add-tree pre-reduces the 8 es tiles to 1
                    # (tile sums partial-sum the m axis), interleaved with
                    # the score matmuls so it finishes right after the
                    # last exp; the rowsum matmul then streams 1 tile.
                    e2 = [espool.tile([P, 512], BF16, tag="esum", bufs=6,
                                      name=f"e2_{ch}_{i}_{k}")
                          for k in range(4)]
                    e4 = [espool.tile([P, 512], BF16, tag="esum2", bufs=4,
                                      name=f"e4_{ch}_{i}_{k}")
                          for k in range(2)]
                    etot = espool.tile([P, 512], BF16, tag="esum3", bufs=2,
                                       name=f"etot_{ch}_{i}")
                    for mb in range(NB):
                        ps = psmm.tile([P, 512], FP32, tag="mm",
                                       name=f"s_ps{ch}_{i}_{mb}")
                        for h in range(H):
                            nc.tensor.matmul(
                                ps[:],
                                kt[h][:, mb * P:(mb + 1) * P],
                                qhat[i][h][:, nsl],
                                start=(h == 0), stop=(h == H - 1),
                            )
                        nc.scalar.activation(es[mb][:], ps[:], AF.Exp)
                        if mb % 2 == 1:
                            k = mb // 2
                            nc.vector.tensor_add(e2[k][:], es[mb - 1][:],
                                                 es[mb][:])
                            if k % 2 == 1:
                                nc.vector.tensor_add(e4[k // 2][:],
                                                     e2[k - 1][:], e2[k][:])
                    nc.vector.tensor_add(etot[:], e4[0][:], e4[1][:])
                    # rowsum (ones lhsT broadcasts the sum to all 128
                    # partitions), then attn@v, accumulated over m blocks
                    nc.tensor.matmul(ps_rs, ones_m[:], etot[:],
                                     start=True, stop=True)
                    for mb in range(NB):
                        nc.tensor.matmul(
                            ps_o, v_sb[mb][:, i * P:(i + 1) * P], es[mb][:],
                            start=(mb == 0), stop=(mb == NB - 1),
                        )
                    rec = outpool.tile([P, 512], FP32, tag="rec",
                                       name=f"rec{ch}_{i}")
                    nc.vector.reciprocal_approx_fast(rec[:], ps_rs)
                    nc.vector.tensor_mul(onorm[i][:, nsl], ps_o, rec[:])

            # ---- output projection + bias (emitted last; the scheduler
            # backfills these matmuls into PE bubbles once a chunk's four
            # heads are normalized) ----
            # nb7 (the critical last chain) adds bias via a 5th K=1
            # accumulation step (ones-row (x) b_proj) and a PSUM->SBUF
            # copy on ACT, which sits idle at the tail; all other nb use
            # DVE tensor_add.  Mid-stream bias matmuls are not worth it:
            # they spend bottleneck PE cycles to relieve DVE slack.
            for nb in range(NB):
                ps = psmm.tile([P, 512], FP32, tag="mm", name=f"p_ps{nb}")
                use_act = (nb == NB - 1)
                for ib in range(H):
                    nc.tensor.matmul(
                        ps[:],
                        onorm[ib][:, nb * P:(nb + 1) * P],
                        wp_sb[ib][:],
                        start=(ib == 0),
                        stop=(ib == H - 1 and not use_act),
                    )
                if use_act:
                    nc.tensor.matmul(ps[:], ones_m[0:1, :], bpr[0:1, :],
                                     start=False, stop=True)
                osb = outpool.tile([P, 512], BF16, tag="osb",
                                   name=f"osb{nb}")
                if use_act:
                    nc.scalar.copy(osb[:], ps[:])
                else:
                    nc.vector.tensor_add(osb[:], ps[:], bpr[:])
                nc.sync.dma_start(out[nb * P:(nb + 1) * P, :], osb[:])


    nc.compile()
    return nc


def _pack(a):
    """[C, W] -> [128, CB*W] SBUF image: partition-block cb at offset cb*W."""
    Crows, W = a.shape
    return np.ascontiguousarray(
        a.reshape(Crows // P, P, W).transpose(1, 0, 2).reshape(P, -1)
    )


def make_in_maps(x, w_qkv, w_proj, b_proj, w_main, w_rest):
    M = _mix_matrix_np(np.asarray(w_main), np.asarray(w_rest))
    bf = ml_dtypes.bfloat16
    wqkvT = np.ascontiguousarray(np.asarray(w_qkv, np.float32).T).astype(bf)
    wpT = np.ascontiguousarray(np.asarray(w_proj, np.float32).T).astype(bf)
    bprow = np.broadcast_to(
        np.asarray(b_proj, np.float32).reshape(1, C), (P, C)
    ).astype(bf)
    qs = np.empty((P, H * H), np.float32)
    for i in range(H):
        for h in range(H):
            qs[:, i * H + h] = np.float32(M[i, h] * SCALE)
    x = np.asarray(x, np.float32)

    # wq chunks: jb-major (8 output 128-col blocks of Q|K), cb-minor
    qk = wqkvT[:, 0:2 * C]                     # [512, 1024]
    wq_chunks = []
    for jb in range(2 * H):
        blk = qk[:, jb * P:(jb + 1) * P]       # [512, 128]
        wq_chunks.append(_pack(blk))           # [128, 512]
    wq0_p = np.ascontiguousarray(wq_chunks[0])
    wqr_p = np.ascontiguousarray(np.concatenate(wq_chunks[1:], axis=1))

    wv_p = _pack(wqkvT[:, 2 * C:3 * C])
    wp_p = _pack(wpT)
    in_maps = []
    for b in range(B):
        xT = np.ascontiguousarray(x[b].T).astype(bf)   # [512, 1024]
        xt_c = [_pack(np.ascontiguousarray(xT[:, ch * 512:(ch + 1) * 512]))
                for ch in range(NCH)]                  # each [128, 2048]
        in_maps.append({
            "wq0": wq0_p,
            "wqr": wqr_p,
            "xt0": xt_c[0],
            "xt1": xt_c[1],
            "wv": wv_p,
            "wpTp": wp_p,
            "bprow": bprow,
            "qscales": qs,
        })
    return in_maps


_NC_CACHE = {}


def get_graph():
    if "nc" not in _NC_CACHE:
        _NC_CACHE["nc"] = build_graph()
    return _NC_CACHE["nc"]


def kernel(x, w_qkv, w_proj, b_proj, w_main, w_rest, _trace=False, _trace_kwargs=None):
    nc = get_graph()
    in_maps = make_in_maps(x, w_qkv, w_proj, b_proj, w_main, w_rest)
    kw = {}
    if _trace:
        kw = {"trace": True}
        if _trace_kwargs:
            kw.update(_trace_kwargs)
    res = run_bass_kernel_spmd(nc, in_maps, core_ids=list(range(NCORES)), **kw)
    outb = np.stack([np.asarray(res.results[i]["out"], dtype=np.float32)
                     for i in range(NCORES)], axis=0)
    if _trace:
        return outb, res
    return outb


# revision 26
# speedup vs baseline: 1.0011x; 1.0011x over previous
"""Trainium2 Bass kernel for mixed-head attention (CIM attention).

Reference computation (per batch element b):
    qkv  = x @ w_qkv.T                                  [N, 3C]
    q,k,v split into H=4 heads of HD=128
    S_h  = (q_h @ k_h.T) * SCALE                        [N, N] per head
    S'_i = sum_h M[i,h] * S_h        (CIM head mix)
    A_i  = softmax(S'_i, axis=-1)
    O_i  = A_i @ v_i
    out  = concat_i(O_i) @ w_proj.T + b_proj

Distribution: data-parallel over B=8, one batch element per NeuronCore.
No collectives needed; host shards/gathers.

Single-core algorithm (all matmuls bf16 with fp32 PSUM accumulation):
  - Host ships x^T, w_qkv^T, w_proj^T pre-transposed, pre-cast to bf16 and
    pre-packed into flat "SBUF image" layouts, so every load is one
    contiguous DMA and the contraction dim is always on SBUF partitions.
    No device transposes anywhere.
  - Concurrent DMAs share HBM bandwidth round-robin per descriptor (not
    FIFO), so the ramp-critical loads (wq jb0 block, x^T ch0 halves, the
    remaining QK weights) are the only transfers in flight at the start;
    everything else (x^T ch1, wv, w_proj, bias) is issued from the scalar
    engine's program *between* the first epilogue copies, which delays
    those DIRECT2Ds until the QK phase is underway.
  - The PE p-state ramps 0.65 -> 1.2 -> 2.4 GHz with sustained activity
    (~2x slower matmuls for the first ~4-9us).  Eight dummy matmuls over
    memset data burn the ramp while the first input DMAs are in flight,
    so real chains start at high clock.
  - The CIM mix is folded into Q: Qhat_i[(h,d), n] = M[i,h]*SCALE*Q_h[d, n].
    Each Q projection tile is cast once PSUM->SBUF (ACT), then scaled into
    the 4 i-variants on DVE (bf16 fast mode, per-partition scalar).  The
    score matmul then contracts over all 512 (h,d) pairs:
    S'_i^T[m, n] = sum_{(h,d)} K[(h,d), m] * Qhat_i[(h,d), n].
  - Scores live in [m_part, n_free] ("S^T") layout so exp is elementwise and
    attn@v needs no transpose: O_i^T[d, n] = sum_m V[m, d] * expS_i^T[m, n].
  - Softmax normalization is deferred past attn@v.  The denominators come
    from a DVE add-tree that pre-reduces the 8 exp tiles to 1 (tile sums
    partial-sum the m axis), then one ones[128,128] stationary matmul whose
    M=128 output broadcasts the rowsum to all partitions for free;
    1/rowsum via reciprocal_approx_fast, applied to O^T with tensor_mul.
  - proj: out[n, c] = sum_{(i,d)} Onorm_i^T[(i,d), n] * w_proj^T[(i,d), c],
    emitted last so the scheduler backfills its matmuls into PE bubbles;
    b_proj is added during the PSUM->SBUF output copy from a
    host-pre-broadcast [128, C] bias tile.  Output is stored bf16 (halves
    the output DMA) and upcast on host.
"""

import os
import sys

for _p in ("/opt/trn_rl_repo",):
    if os.path.isdir(_p) and _p not in sys.path:
        sys.path.insert(0, _p)

import numpy as np
import ml_dtypes

import concourse.bass as bass
import concourse.tile as tile
from concourse import bacc, mybir
from concourse.bass_utils import run_bass_kernel_spmd

B, N, C, H = 8, 1024, 512, 4
HD = C // H          # 128
SCALE = HD ** -0.5
NCORES = 8
P = 128              # partitions
NCH = N // 512       # 512-wide free-dim chunks per N
NB = N // P          # 128-row blocks per N
CB = C // P          # 128-row blocks per C

BF16 = mybir.dt.bfloat16
FP32 = mybir.dt.float32
AF = mybir.ActivationFunctionType


def _mix_matrix_np(w_main: np.ndarray, w_rest: np.ndarray) -> np.ndarray:
    rows = np.repeat(np.arange(H), H - 1)
    cols = np.array([[j for j in range(H) if j != i] for i in range(H)]).ravel()
    M = np.zeros((H, H), dtype=np.float64)
    M[rows, cols] = w_rest.astype(np.float64).ravel()
    M += np.diag(w_main.astype(np.float64))
    return M


def build_graph():
    nc = bacc.Bacc(
        "TRN2",
        target_bir_lowering=False,
        debug=False,
        num_devices=NCORES,
    )

    # Priority-chunked input layouts.
    # wq0/wqr: w_qkv^T Q+K columns packed jb-major (jb = 8 output 128-col
    #   blocks: Q heads 0-3 then K heads 0-3), cb-minor: chunk jb holds the
    #   four [128,128] cb-blocks side by side.
    # xt0/xt1: x^T packed ch-major (ch = 512-wide n chunk), cb-minor: chunk
    #   ch holds four [128,512] cb-blocks side by side.
    wq0 = nc.dram_tensor("wq0", [P, CB * P], BF16, kind="ExternalInput").ap()
    wqr = nc.dram_tensor("wqr", [P, 7 * CB * P], BF16, kind="ExternalInput").ap()
    xt0 = nc.dram_tensor("xt0", [P, CB * 512], BF16, kind="ExternalInput").ap()
    xt1 = nc.dram_tensor("xt1", [P, CB * 512], BF16, kind="ExternalInput").ap()
    wv = nc.dram_tensor("wv", [P, CB * C], BF16, kind="ExternalInput").ap()
    wpTp = nc.dram_tensor("wpTp", [P, CB * C], BF16, kind="ExternalInput").ap()
    bprow = nc.dram_tensor("bprow", [P, C], BF16, kind="ExternalInput").ap()
    qscales = nc.dram_tensor("qscales", [P, H * H], FP32, kind="ExternalInput").ap()
    out = nc.dram_tensor("out", [N, C], BF16, kind="ExternalOutput").ap()

    with tile.TileContext(nc, pool_alloc_mode="queue") as tc:
        with (
            tc.tile_pool(name="const", bufs=1) as cpool,
            tc.tile_pool(name="wts", bufs=1) as wpool,
            tc.tile_pool(name="qkv", bufs=1) as qkvpool,
            tc.tile_pool(name="es", bufs=12) as espool,
            tc.tile_pool(name="onorm", bufs=1) as opool,
            tc.tile_pool(name="outsb", bufs=3) as outpool,
            tc.tile_pool(name="ps2", bufs=2, space="PSUM") as ps2pool,
            tc.tile_pool(name="psmm", bufs=4, space="PSUM") as psmm,
        ):
            # ---- priority-ordered input DMA (issuable engines: sync/SP,
            # scalar/ACT, gpsimd; ~0.7us serial per dma_start on a
            # sequencer, ~1.4us trigger->data latency).
            # Concurrent DMAs share bandwidth round-robin per descriptor
            # (NOT FIFO across transfers), so non-critical loads must not
            # be in flight while the critical Q/K feed streams.  Critical
            # now: wq0+xt0 halves, then wqr.  Everything else (xt1, wv,
            # wpp, bpr) is issued from the scalar engine INTERLEAVED with
            # the first epilogue copies, so those DMAs trigger only once
            # the QK phase is underway (program order on the sequencer
            # delays them past the copies' semaphore waits).
            warm = cpool.tile([P, 512], BF16, tag="warm")
            nc.gpsimd.memset(warm[:], 0.0)
            ones_m = cpool.tile([P, P], BF16, tag="ones_m")
            nc.gpsimd.memset(ones_m[:], 1.0)

            # xt0 in four per-cb chunks (completion-event granularity: the
            # first chain's cb-steps unblock as each 128 KB lands instead
            # of waiting a 256 KB half); wqr in three progressive chunks
            # matched to jb consumption order.
            xt0_sb = wpool.tile([P, CB * 512], BF16, tag="xt0", name="xt0")
            wq0_sb = wpool.tile([P, CB * P], BF16, tag="wq0", name="wq0")
            nc.sync.dma_start(xt0_sb[:, 0:512], xt0[:, 0:512])
            nc.scalar.dma_start(wq0_sb[:], wq0[:, :])
            nc.sync.dma_start(xt0_sb[:, 1024:1536], xt0[:, 1024:1536])
            nc.scalar.dma_start(xt0_sb[:, 512:1024], xt0[:, 512:1024])
            nc.scalar.dma_start(xt0_sb[:, 1536:2048], xt0[:, 1536:2048])

            wqr_sb = wpool.tile([P, 7 * CB * P], BF16, tag="wqr", name="wqr")
            nc.sync.dma_start(wqr_sb[:, 0:1024], wqr[:, 0:1024])
            nc.sync.dma_start(wqr_sb[:, 1024:2048], wqr[:, 1024:2048])
            nc.sync.dma_start(wqr_sb[:, 2048:3584], wqr[:, 2048:3584])
            qsc = cpool.tile([P, H * H], FP32, tag="qsc")
            nc.scalar.dma_start(qsc[:], qscales[:, :])

            # allocated here, loaded from inside the QKV loop (below)
            xt1_sb = wpool.tile([P, CB * 512], BF16, tag="xt1", name="xt1")
            wvp = wpool.tile([P, CB * C], BF16, tag="wvp", name="wvp")
            wpp = wpool.tile([P, CB * C], BF16, tag="wpp", name="wpp")
            bpr = cpool.tile([P, C], BF16, tag="bpr")

            def w_qk(jb, cb):
                if jb == 0:
                    return wq0_sb[:, cb * P:(cb + 1) * P]
                return wqr_sb[:, (jb - 1) * CB * P + cb * P:
                              (jb - 1) * CB * P + (cb + 1) * P]

            def xt(ch, cb):
                t = xt0_sb if ch == 0 else xt1_sb
                return t[:, cb * 512:(cb + 1) * 512]

            def xt_mb(cb, mb):
                # [128,128] m-block mb of cb-block cb (V projection lhsT)
                ch, j = divmod(mb, CB)
                t = xt0_sb if ch == 0 else xt1_sb
                return t[:, cb * 512 + j * P:cb * 512 + (j + 1) * P]

            wv_sb = [wvp[:, cb * C:(cb + 1) * C] for cb in range(CB)]
            wp_sb = [wpp[:, cb * C:(cb + 1) * C] for cb in range(CB)]

            # ---- QKV projections ----
            # qhat[i][h]: [128(d), N] bf16 ; kt[h]: [128(d), N] ; v[mb]: [128(m), C]
            qhat = [[qkvpool.tile([P, N], BF16, tag=f"qhat{i}_{h}",
                                  name=f"qhat{i}_{h}")
                     for h in range(H)] for i in range(H)]
            kt = [qkvpool.tile([P, N], BF16, tag=f"kt{h}", name=f"kt{h}")
                  for h in range(H)]
            v_sb = [qkvpool.tile([P, C], BF16, tag=f"v{mb}", name=f"v{mb}")
                    for mb in range(NB)]

            # PE p-state warmup: the tensor engine ramps 0.65 -> 1.2 -> 2.4
            # GHz with sustained activity (~2x slower matmuls for the first
            # ~9us of PE busy).  Burn that ramp on dummy matmuls over
            # memset data while the first input DMAs are still in flight,
            # so the real chains start at high clock.
            # 6 dummies end right as the first real operands land (~10.5us);
            # more would delay real work, fewer would leave the first
            # (data-stalled anyway) real chains at mid p-state
            ps_warm = psmm.tile([P, 512], FP32, tag="mm", name="warm_ps")
            for w in range(6):
                nc.tensor.matmul(ps_warm[:], ones_m[:], warm[:],
                                 start=True, stop=True)

            # Q and K: one [128, 512] chain per (ch, jb) so the whole ch=0
            # half runs off the first x^T chunk (xt1 lands while ch=0
            # computes); per-chain epilogue copy (ACT) + scaled qhat
            # variants (DVE).
            for ch in range(NCH):
                csl = slice(ch * 512, (ch + 1) * 512)
                for jb in range(2 * H):      # 0-3: Q heads, 4-7: K heads
                    ps = psmm.tile([P, 512], FP32, tag="mm",
                                   name=f"qk_ps{ch}_{jb}")
                    for cb in range(CB):
                        nc.tensor.matmul(
                            ps[:],
                            w_qk(jb, cb),
                            xt(ch, cb),
                            start=(cb == 0), stop=(cb == CB - 1),
                        )
                    if jb < H:
                        h = jb
                        # PSUM->SBUF cast on ACT, then 4 cheap bf16 scaled
                        # copies on DVE (CIM mix scales folded into qhat)
                        qb = qkvpool.tile([P, 512], BF16, tag="qb", bufs=8,
                                          name=f"qb{ch}_{h}")
                        nc.scalar.copy(qb[:], ps[:])
                        for i in range(H):
                            sc = qsc[:, i * H + h:i * H + h + 1]
                            nc.vector.tensor_scalar_mul(
                                qhat[i][h][:, csl], qb[:], sc)
                    else:
                        h = jb - H
                        nc.scalar.copy(kt[h][:, csl], ps[:])
                    if ch == 0:
                        # deferred non-critical loads: the DIRECT2D sits
                        # after this epilogue copy in scalar program order,
                        # keeping early HBM bandwidth for the QK feed
                        if jb == 0:
                            nc.scalar.dma_start(xt1_sb[:], xt1[:, :])
                        elif jb == 1:
                            nc.scalar.dma_start(wvp[:], wv[:, :])
                        elif jb == 2:
                            nc.scalar.dma_start(wpp[:], wpTp[:, :])
                        elif jb == 3:
                            nc.scalar.dma_start(bpr[:], bprow[:, :])

            # V: out[m_block, c] = sum_cb xT[cb][:, mblk].T @ wvT[cb]
            for mb in range(NB):
                ps = psmm.tile([P, 512], FP32, tag="mm", name=f"v_ps{mb}")
                for cb in range(CB):
                    nc.tensor.matmul(
                        ps[:],
                        xt_mb(cb, mb),
                        wv_sb[cb][:],
                        start=(cb == 0), stop=(cb == CB - 1),
                    )
                nc.vector.tensor_copy(v_sb[mb][:], ps[:])

            # ---- chunk-outer head loop: scores -> exp -> rowsum/attnv ----
            onorm = [opool.tile([P, N], BF16, tag=f"onorm{i}", name=f"onorm{i}")
                     for i in range(H)]

            for ch in range(NCH):
                nsl = slice(ch * 512, (ch + 1) * 512)
                for i in range(H):
                    es = [espool.tile([P, 512], BF16, tag="es",
                                      name=f"es{ch}_{i}_{mb}")
                          for mb in range(NB)]
                    ps_rso = ps2pool.tile([P, N], FP32, tag="mm2",
                                          name=f"rso{ch}_{i}")
                    ps_rs = ps_rso[:, 0:512]
                    ps_o = ps_rso[:, 512:1024]
                    # DVE running sum pre-reduces the 8 es tiles to 1
                    # (tile sums partial-sum the m axis).  A running sum
                    # (not a pairwise tree) so the LAST exp feeds exactly
                    # ONE add before the rowsum matmul — the tree's
                    # 3-serial-add suffix after exp(mb7) sat directly on
                    # the final group's exec-ending chain.
                    ss = [espool.tile([P, 512], BF16, tag="esum", bufs=3,
                                      name=f"ss_{ch}_{i}_{k}")
                          for k in range(NB - 1)]
                    for mb in range(NB):
                        ps = psmm.tile([P, 512], FP32, tag="mm",
                                       name=f"s_ps{ch}_{i}_{mb}")
                        for h in range(H):
                            nc.tensor.matmul(
                                ps[:],
                                kt[h][:, mb * P:(mb + 1) * P],
                                qhat[i][h][:, nsl],
                                start=(h == 0), stop=(h == H - 1),
                            )
                        nc.scalar.activation(es[mb][:], ps[:], AF.Exp)
                        if mb == 1:
                            nc.vector.tensor_add(ss[0][:], es[0][:],
                                                 es[1][:])
                        elif mb >= 2:
                            nc.vector.tensor_add(ss[mb - 1][:],
                                                 ss[mb - 2][:], es[mb][:])
                    # rowsum (ones lhsT broadcasts the sum to all 128
                    # partitions), then attn@v, accumulated over m blocks
                    nc.tensor.matmul(ps_rs, ones_m[:], ss[NB - 2][:],
                                     start=True, stop=True)
                    for mb in range(NB):
                        nc.tensor.matmul(
                            ps_o, v_sb[mb][:, i * P:(i + 1) * P], es[mb][:],
                            start=(mb == 0), stop=(mb == NB - 1),
                        )
                    rec = outpool.tile([P, 512], FP32, tag="rec",
                                       name=f"rec{ch}_{i}")
                    nc.vector.reciprocal_approx_fast(rec[:], ps_rs)
                    nc.vector.tensor_mul(onorm[i][:, nsl], ps_o, rec[:])

            # ---- output projection + bias (emitted last; the scheduler
            # backfills these matmuls into PE bubbles once a chunk's four
            # heads are normalized) ----
            # nb7 (the critical last chain) adds bias via a 5th K=1
            # accumulation step (ones-row (x) b_proj) and a PSUM->SBUF
            # copy on ACT, which sits idle at the tail; all other nb use
            # DVE tensor_add.  Mid-stream bias matmuls are not worth it:
            # they spend bottleneck PE cycles to relieve DVE slack.
            for nb in range(NB):
                ps = psmm.tile([P, 512], FP32, tag="mm", name=f"p_ps{nb}")
                use_act = (nb == NB - 1)
                for ib in range(H):
                    nc.tensor.matmul(
                        ps[:],
                        onorm[ib][:, nb * P:(nb + 1) * P],
                        wp_sb[ib][:],
                        start=(ib == 0),
                        stop=(ib == H - 1 and not use_act),
                    )
                if use_act:
                    nc.tensor.matmul(ps[:], ones_m[0:1, :], bpr[0:1, :],
                                     start=False, stop=True)
                osb = outpool.tile([P, 512], BF16, tag="osb",
                                   name=f"osb{nb}")
                if use_act:
                    nc.scalar.copy(osb[:], ps[:])
                else:
                    nc.vector.tensor_add(osb[:], ps[:], bpr[:])
                nc.sync.dma_start(out[nb * P:(nb + 1) * P, :], osb[:])


    nc.compile()
    return nc


def _pack(a):
    """[C, W] -> [128, CB*W] SBUF image: partition-block cb at offset cb*W."""
    Crows, W = a.shape
    return np.ascontiguousarray(
        a.reshape(Crows // P, P, W).transpose(1, 0, 2).reshape(P, -1)
    )


def make_in_maps(x, w_qkv, w_proj, b_proj, w_main, w_rest):
    M = _mix_matrix_np(np.asarray(w_main), np.asarray(w_rest))
    bf = ml_dtypes.bfloat16
    wqkvT = np.ascontiguousarray(np.asarray(w_qkv, np.float32).T).astype(bf)
    wpT = np.ascontiguousarray(np.asarray(w_proj, np.float32).T).astype(bf)
    bprow = np.broadcast_to(
        np.asarray(b_proj, np.float32).reshape(1, C), (P, C)
    ).astype(bf)
    qs = np.empty((P, H * H), np.float32)
    for i in range(H):
        for h in range(H):
            qs[:, i * H + h] = np.float32(M[i, h] * SCALE)
    x = np.asarray(x, np.float32)

    # wq chunks: jb-major (8 output 128-col blocks of Q|K), cb-minor
    qk = wqkvT[:, 0:2 * C]                     # [512, 1024]
    wq_chunks = []
    for jb in range(2 * H):
        blk = qk[:, jb * P:(jb + 1) * P]       # [512, 128]
        wq_chunks.append(_pack(blk))           # [128, 512]
    wq0_p = np.ascontiguousarray(wq_chunks[0])
    wqr_p = np.ascontiguousarray(np.concatenate(wq_chunks[1:], axis=1))

    wv_p = _pack(wqkvT[:, 2 * C:3 * C])
    wp_p = _pack(wpT)
    in_maps = []
    for b in range(B):
        xT = np.ascontiguousarray(x[b].T).astype(bf)   # [512, 1024]
        xt_c = [_pack(np.ascontiguousarray(xT[:, ch * 512:(ch + 1) * 512]))
                for ch in range(NCH)]                  # each [128, 2048]
        in_maps.append({
            "wq0": wq0_p,
            "wqr": wqr_p,
            "xt0": xt_c[0],
            "xt1": xt_c[1],
            "wv": wv_p,
            "wpTp": wp_p,
            "bprow": bprow,
            "qscales": qs,
        })
    return in_maps


_NC_CACHE = {}


def get_graph():
    if "nc" not in _NC_CACHE:
        _NC_CACHE["nc"] = build_graph()
    return _NC_CACHE["nc"]


def kernel(x, w_qkv, w_proj, b_proj, w_main, w_rest, _trace=False, _trace_kwargs=None):
    nc = get_graph()
    in_maps = make_in_maps(x, w_qkv, w_proj, b_proj, w_main, w_rest)
    kw = {}
    if _trace:
        kw = {"trace": True}
        if _trace_kwargs:
            kw.update(_trace_kwargs)
    res = run_bass_kernel_spmd(nc, in_maps, core_ids=list(range(NCORES)), **kw)
    outb = np.stack([np.asarray(res.results[i]["out"], dtype=np.float32)
                     for i in range(NCORES)], axis=0)
    if _trace:
        return outb, res
    return outb


# revision 27
# speedup vs baseline: 1.0046x; 1.0035x over previous
"""Trainium2 Bass kernel for mixed-head attention (CIM attention).

Reference computation (per batch element b):
    qkv  = x @ w_qkv.T                                  [N, 3C]
    q,k,v split into H=4 heads of HD=128
    S_h  = (q_h @ k_h.T) * SCALE                        [N, N] per head
    S'_i = sum_h M[i,h] * S_h        (CIM head mix)
    A_i  = softmax(S'_i, axis=-1)
    O_i  = A_i @ v_i
    out  = concat_i(O_i) @ w_proj.T + b_proj

Distribution: data-parallel over B=8, one batch element per NeuronCore.
No collectives needed; host shards/gathers.

Single-core algorithm (all matmuls bf16 with fp32 PSUM accumulation):
  - Host ships x^T, w_qkv^T, w_proj^T pre-transposed, pre-cast to bf16 and
    pre-packed into flat "SBUF image" layouts, so every load is one
    contiguous DMA and the contraction dim is always on SBUF partitions.
    No device transposes anywhere.
  - Concurrent DMAs share HBM bandwidth round-robin per descriptor (not
    FIFO), so the ramp-critical loads (wq jb0 block, x^T ch0 halves, the
    remaining QK weights) are the only transfers in flight at the start;
    everything else (x^T ch1, wv, w_proj, bias) is issued from the scalar
    engine's program *between* the first epilogue copies, which delays
    those DIRECT2Ds until the QK phase is underway.
  - The PE p-state ramps 0.65 -> 1.2 -> 2.4 GHz with sustained activity
    (~2x slower matmuls for the first ~4-9us).  Eight dummy matmuls over
    memset data burn the ramp while the first input DMAs are in flight,
    so real chains start at high clock.
  - The CIM mix is folded into Q: Qhat_i[(h,d), n] = M[i,h]*SCALE*Q_h[d, n].
    Each Q projection tile is cast once PSUM->SBUF (ACT), then scaled into
    the 4 i-variants on DVE (bf16 fast mode, per-partition scalar).  The
    score matmul then contracts over all 512 (h,d) pairs:
    S'_i^T[m, n] = sum_{(h,d)} K[(h,d), m] * Qhat_i[(h,d), n].
  - Scores live in [m_part, n_free] ("S^T") layout so exp is elementwise and
    attn@v needs no transpose: O_i^T[d, n] = sum_m V[m, d] * expS_i^T[m, n].
  - Softmax normalization is deferred past attn@v.  The denominators come
    from a DVE add-tree that pre-reduces the 8 exp tiles to 1 (tile sums
    partial-sum the m axis), then one ones[128,128] stationary matmul whose
    M=128 output broadcasts the rowsum to all partitions for free;
    1/rowsum via reciprocal_approx_fast, applied to O^T with tensor_mul.
  - proj: out[n, c] = sum_{(i,d)} Onorm_i^T[(i,d), n] * w_proj^T[(i,d), c],
    emitted last so the scheduler backfills its matmuls into PE bubbles;
    b_proj is added during the PSUM->SBUF output copy from a
    host-pre-broadcast [128, C] bias tile.  Output is stored bf16 (halves
    the output DMA) and upcast on host.
"""

import os
import sys

for _p in ("/opt/trn_rl_repo",):
    if os.path.isdir(_p) and _p not in sys.path:
        sys.path.insert(0, _p)

import numpy as np
import ml_dtypes

import concourse.bass as bass
import concourse.tile as tile
from concourse import bacc, mybir
from concourse.bass_utils import run_bass_kernel_spmd

B, N, C, H = 8, 1024, 512, 4
HD = C // H          # 128
SCALE = HD ** -0.5
NCORES = 8
P = 128              # partitions
NCH = N // 512       # 512-wide free-dim chunks per N
NB = N // P          # 128-row blocks per N
CB = C // P          # 128-row blocks per C

BF16 = mybir.dt.bfloat16
FP32 = mybir.dt.float32
AF = mybir.ActivationFunctionType


def _mix_matrix_np(w_main: np.ndarray, w_rest: np.ndarray) -> np.ndarray:
    rows = np.repeat(np.arange(H), H - 1)
    cols = np.array([[j for j in range(H) if j != i] for i in range(H)]).ravel()
    M = np.zeros((H, H), dtype=np.float64)
    M[rows, cols] = w_rest.astype(np.float64).ravel()
    M += np.diag(w_main.astype(np.float64))
    return M


def build_graph():
    nc = bacc.Bacc(
        "TRN2",
        target_bir_lowering=False,
        debug=False,
        num_devices=NCORES,
    )

    # Priority-chunked input layouts.
    # wq0/wqr: w_qkv^T Q+K columns packed jb-major (jb = 8 output 128-col
    #   blocks: Q heads 0-3 then K heads 0-3), cb-minor: chunk jb holds the
    #   four [128,128] cb-blocks side by side.
    # xt0/xt1: x^T packed ch-major (ch = 512-wide n chunk), cb-minor: chunk
    #   ch holds four [128,512] cb-blocks side by side.
    wq0 = nc.dram_tensor("wq0", [P, CB * P], BF16, kind="ExternalInput").ap()
    wqr = nc.dram_tensor("wqr", [P, 7 * CB * P], BF16, kind="ExternalInput").ap()
    xt0 = nc.dram_tensor("xt0", [P, CB * 512], BF16, kind="ExternalInput").ap()
    xt1 = nc.dram_tensor("xt1", [P, CB * 512], BF16, kind="ExternalInput").ap()
    wv = nc.dram_tensor("wv", [P, CB * C], BF16, kind="ExternalInput").ap()
    wpTp = nc.dram_tensor("wpTp", [P, CB * C], BF16, kind="ExternalInput").ap()
    bprow = nc.dram_tensor("bprow", [P, C], BF16, kind="ExternalInput").ap()
    qscales = nc.dram_tensor("qscales", [P, H * H], FP32, kind="ExternalInput").ap()
    out = nc.dram_tensor("out", [N, C], BF16, kind="ExternalOutput").ap()

    with tile.TileContext(nc, pool_alloc_mode="queue") as tc:
        with (
            tc.tile_pool(name="const", bufs=1) as cpool,
            tc.tile_pool(name="wts", bufs=1) as wpool,
            tc.tile_pool(name="qkv", bufs=1) as qkvpool,
            tc.tile_pool(name="es", bufs=12) as espool,
            tc.tile_pool(name="onorm", bufs=1) as opool,
            tc.tile_pool(name="outsb", bufs=3) as outpool,
            tc.tile_pool(name="ps2", bufs=2, space="PSUM") as ps2pool,
            tc.tile_pool(name="psmm", bufs=4, space="PSUM") as psmm,
        ):
            # ---- priority-ordered input DMA (issuable engines: sync/SP,
            # scalar/ACT, gpsimd; ~0.7us serial per dma_start on a
            # sequencer, ~1.4us trigger->data latency).
            # Concurrent DMAs share bandwidth round-robin per descriptor
            # (NOT FIFO across transfers), so non-critical loads must not
            # be in flight while the critical Q/K feed streams.  Critical
            # now: wq0+xt0 halves, then wqr.  Everything else (xt1, wv,
            # wpp, bpr) is issued from the scalar engine INTERLEAVED with
            # the first epilogue copies, so those DMAs trigger only once
            # the QK phase is underway (program order on the sequencer
            # delays them past the copies' semaphore waits).
            warm = cpool.tile([P, 512], BF16, tag="warm")
            nc.gpsimd.memset(warm[:], 0.0)
            ones_m = cpool.tile([P, P], BF16, tag="ones_m")
            nc.gpsimd.memset(ones_m[:], 1.0)

            # xt0 in four per-cb chunks (completion-event granularity: the
            # first chain's cb-steps unblock as each 128 KB lands instead
            # of waiting a 256 KB half); wqr in three progressive chunks
            # matched to jb consumption order.
            xt0_sb = wpool.tile([P, CB * 512], BF16, tag="xt0", name="xt0")
            wq0_sb = wpool.tile([P, CB * P], BF16, tag="wq0", name="wq0")
            nc.sync.dma_start(xt0_sb[:, 0:512], xt0[:, 0:512])
            nc.scalar.dma_start(wq0_sb[:], wq0[:, :])
            nc.sync.dma_start(xt0_sb[:, 1024:1536], xt0[:, 1024:1536])
            nc.scalar.dma_start(xt0_sb[:, 512:1024], xt0[:, 512:1024])
            nc.scalar.dma_start(xt0_sb[:, 1536:2048], xt0[:, 1536:2048])

            wqr_sb = wpool.tile([P, 7 * CB * P], BF16, tag="wqr", name="wqr")
            nc.sync.dma_start(wqr_sb[:, 0:1024], wqr[:, 0:1024])
            nc.sync.dma_start(wqr_sb[:, 1024:2048], wqr[:, 1024:2048])
            nc.sync.dma_start(wqr_sb[:, 2048:3584], wqr[:, 2048:3584])
            qsc = cpool.tile([P, H * H], FP32, tag="qsc")
            nc.scalar.dma_start(qsc[:], qscales[:, :])

            # allocated here, loaded from inside the QKV loop (below)
            xt1_sb = wpool.tile([P, CB * 512], BF16, tag="xt1", name="xt1")
            wvp = wpool.tile([P, CB * C], BF16, tag="wvp", name="wvp")
            wpp = wpool.tile([P, CB * C], BF16, tag="wpp", name="wpp")
            bpr = cpool.tile([P, C], BF16, tag="bpr")

            def w_qk(jb, cb):
                if jb == 0:
                    return wq0_sb[:, cb * P:(cb + 1) * P]
                return wqr_sb[:, (jb - 1) * CB * P + cb * P:
                              (jb - 1) * CB * P + (cb + 1) * P]

            def xt(ch, cb):
                t = xt0_sb if ch == 0 else xt1_sb
                return t[:, cb * 512:(cb + 1) * 512]

            def xt_mb(cb, mb):
                # [128,128] m-block mb of cb-block cb (V projection lhsT)
                ch, j = divmod(mb, CB)
                t = xt0_sb if ch == 0 else xt1_sb
                return t[:, cb * 512 + j * P:cb * 512 + (j + 1) * P]

            wv_sb = [wvp[:, cb * C:(cb + 1) * C] for cb in range(CB)]
            wp_sb = [wpp[:, cb * C:(cb + 1) * C] for cb in range(CB)]

            # ---- QKV projections ----
            # qhat[i][h]: [128(d), N] bf16 ; kt[h]: [128(d), N] ; v[mb]: [128(m), C]
            qhat = [[qkvpool.tile([P, N], BF16, tag=f"qhat{i}_{h}",
                                  name=f"qhat{i}_{h}")
                     for h in range(H)] for i in range(H)]
            kt = [qkvpool.tile([P, N], BF16, tag=f"kt{h}", name=f"kt{h}")
                  for h in range(H)]
            v_sb = [qkvpool.tile([P, C], BF16, tag=f"v{mb}", name=f"v{mb}")
                    for mb in range(NB)]

            # PE p-state warmup: the tensor engine ramps 0.65 -> 1.2 -> 2.4
            # GHz with sustained activity (~2x slower matmuls for the first
            # ~9us of PE busy).  Burn that ramp on dummy matmuls over
            # memset data while the first input DMAs are still in flight,
            # so the real chains start at high clock.
            # 6 dummies end right as the first real operands land (~10.5us);
            # more would delay real work, fewer would leave the first
            # (data-stalled anyway) real chains at mid p-state
            ps_warm = psmm.tile([P, 512], FP32, tag="mm", name="warm_ps")
            for w in range(6):
                nc.tensor.matmul(ps_warm[:], ones_m[:], warm[:],
                                 start=True, stop=True)

            # Q and K: one [128, 512] chain per (ch, jb) so the whole ch=0
            # half runs off the first x^T chunk (xt1 lands while ch=0
            # computes); per-chain epilogue copy (ACT) + scaled qhat
            # variants (DVE).
            for ch in range(NCH):
                csl = slice(ch * 512, (ch + 1) * 512)
                for jb in range(2 * H):      # 0-3: Q heads, 4-7: K heads
                    ps = psmm.tile([P, 512], FP32, tag="mm",
                                   name=f"qk_ps{ch}_{jb}")
                    for cb in range(CB):
                        nc.tensor.matmul(
                            ps[:],
                            w_qk(jb, cb),
                            xt(ch, cb),
                            start=(cb == 0), stop=(cb == CB - 1),
                        )
                    if jb < H:
                        h = jb
                        # PSUM->SBUF cast on ACT, then 4 cheap bf16 scaled
                        # copies on DVE (CIM mix scales folded into qhat)
                        qb = qkvpool.tile([P, 512], BF16, tag="qb", bufs=8,
                                          name=f"qb{ch}_{h}")
                        nc.scalar.copy(qb[:], ps[:])
                        for i in range(H):
                            sc = qsc[:, i * H + h:i * H + h + 1]
                            nc.vector.tensor_scalar_mul(
                                qhat[i][h][:, csl], qb[:], sc)
                    else:
                        h = jb - H
                        nc.scalar.copy(kt[h][:, csl], ps[:])
                    if ch == 0:
                        # deferred non-critical loads: the DIRECT2D sits
                        # after this epilogue copy in scalar program order,
                        # keeping early HBM bandwidth for the QK feed
                        if jb == 0:
                            nc.scalar.dma_start(xt1_sb[:], xt1[:, :])
                        elif jb == 1:
                            nc.scalar.dma_start(wvp[:], wv[:, :])
                        elif jb == 2:
                            nc.scalar.dma_start(wpp[:], wpTp[:, :])
                        elif jb == 3:
                            nc.scalar.dma_start(bpr[:], bprow[:, :])

            # V: out[m_block, c] = sum_cb xT[cb][:, mblk].T @ wvT[cb]
            for mb in range(NB):
                ps = psmm.tile([P, 512], FP32, tag="mm", name=f"v_ps{mb}")
                for cb in range(CB):
                    nc.tensor.matmul(
                        ps[:],
                        xt_mb(cb, mb),
                        wv_sb[cb][:],
                        start=(cb == 0), stop=(cb == CB - 1),
                    )
                nc.vector.tensor_copy(v_sb[mb][:], ps[:])

            # ---- chunk-outer head loop: scores -> exp -> rowsum/attnv ----
            onorm = [opool.tile([P, N], BF16, tag=f"onorm{i}", name=f"onorm{i}")
                     for i in range(H)]

            for ch in range(NCH):
                nsl = slice(ch * 512, (ch + 1) * 512)
                for i in range(H):
                    es = [espool.tile([P, 512], BF16, tag="es",
                                      name=f"es{ch}_{i}_{mb}")
                          for mb in range(NB)]
                    ps_rso = ps2pool.tile([P, N], FP32, tag="mm2",
                                          name=f"rso{ch}_{i}")
                    ps_rs = ps_rso[:, 0:512]
                    ps_o = ps_rso[:, 512:1024]
                    # DVE add-tree pre-reduces the 8 es tiles to 1
                    # (tile sums partial-sum the m axis), interleaved with
                    # the score matmuls so it finishes right after the
                    # last exp; the rowsum matmul then streams 1 tile.
                    # (A running-sum variant — one add after the last exp
                    # instead of three — measured speed-neutral and
                    # slightly worse numerically; the tree stays.)
                    e2 = [espool.tile([P, 512], BF16, tag="esum", bufs=6,
                                      name=f"e2_{ch}_{i}_{k}")
                          for k in range(4)]
                    e4 = [espool.tile([P, 512], BF16, tag="esum2", bufs=4,
                                      name=f"e4_{ch}_{i}_{k}")
                          for k in range(2)]
                    etot = espool.tile([P, 512], BF16, tag="esum3", bufs=2,
                                       name=f"etot_{ch}_{i}")
                    for mb in range(NB):
                        ps = psmm.tile([P, 512], FP32, tag="mm",
                                       name=f"s_ps{ch}_{i}_{mb}")
                        for h in range(H):
                            nc.tensor.matmul(
                                ps[:],
                                kt[h][:, mb * P:(mb + 1) * P],
                                qhat[i][h][:, nsl],
                                start=(h == 0), stop=(h == H - 1),
                            )
                        nc.scalar.activation(es[mb][:], ps[:], AF.Exp)
                        if mb % 2 == 1:
                            k = mb // 2
                            nc.vector.tensor_add(e2[k][:], es[mb - 1][:],
                                                 es[mb][:])
                            if k % 2 == 1:
                                nc.vector.tensor_add(e4[k // 2][:],
                                                     e2[k - 1][:], e2[k][:])
                    nc.vector.tensor_add(etot[:], e4[0][:], e4[1][:])
                    # rowsum (ones lhsT broadcasts the sum to all 128
                    # partitions), then attn@v, accumulated over m blocks
                    nc.tensor.matmul(ps_rs, ones_m[:], etot[:],
                                     start=True, stop=True)
                    for mb in range(NB):
                        nc.tensor.matmul(
                            ps_o, v_sb[mb][:, i * P:(i + 1) * P], es[mb][:],
                            start=(mb == 0), stop=(mb == NB - 1),
                        )
                    rec = outpool.tile([P, 512], FP32, tag="rec",
                                       name=f"rec{ch}_{i}")
                    nc.vector.reciprocal_approx_fast(rec[:], ps_rs)
                    nc.vector.tensor_mul(onorm[i][:, nsl], ps_o, rec[:])

            # ---- output projection + bias (emitted last; the scheduler
            # backfills these matmuls into PE bubbles once a chunk's four
            # heads are normalized) ----
            # nb7 (the critical last chain) adds bias via a 5th K=1
            # accumulation step (ones-row (x) b_proj) and a PSUM->SBUF
            # copy on ACT, which sits idle at the tail; all other nb use
            # DVE tensor_add.  Mid-stream bias matmuls are not worth it:
            # they spend bottleneck PE cycles to relieve DVE slack.
            for nb in range(NB):
                ps = psmm.tile([P, 512], FP32, tag="mm", name=f"p_ps{nb}")
                use_act = (nb == NB - 1)
                for ib in range(H):
                    nc.tensor.matmul(
                        ps[:],
                        onorm[ib][:, nb * P:(nb + 1) * P],
                        wp_sb[ib][:],
                        start=(ib == 0),
                        stop=(ib == H - 1 and not use_act),
                    )
                if use_act:
                    nc.tensor.matmul(ps[:], ones_m[0:1, :], bpr[0:1, :],
                                     start=False, stop=True)
                osb = outpool.tile([P, 512], BF16, tag="osb",
                                   name=f"osb{nb}")
                if use_act:
                    nc.scalar.copy(osb[:], ps[:])
                else:
                    nc.vector.tensor_add(osb[:], ps[:], bpr[:])
                nc.sync.dma_start(out[nb * P:(nb + 1) * P, :], osb[:])


    nc.compile()
    return nc


def _pack(a):
    """[C, W] -> [128, CB*W] SBUF image: partition-block cb at offset cb*W."""
    Crows, W = a.shape
    return np.ascontiguousarray(
        a.reshape(Crows // P, P, W).transpose(1, 0, 2).reshape(P, -1)
    )


def make_in_maps(x, w_qkv, w_proj, b_proj, w_main, w_rest):
    M = _mix_matrix_np(np.asarray(w_main), np.asarray(w_rest))
    bf = ml_dtypes.bfloat16
    wqkvT = np.ascontiguousarray(np.asarray(w_qkv, np.float32).T).astype(bf)
    wpT = np.ascontiguousarray(np.asarray(w_proj, np.float32).T).astype(bf)
    bprow = np.broadcast_to(
        np.asarray(b_proj, np.float32).reshape(1, C), (P, C)
    ).astype(bf)
    qs = np.empty((P, H * H), np.float32)
    for i in range(H):
        for h in range(H):
            qs[:, i * H + h] = np.float32(M[i, h] * SCALE)
    x = np.asarray(x, np.float32)

    # wq chunks: jb-major (8 output 128-col blocks of Q|K), cb-minor
    qk = wqkvT[:, 0:2 * C]                     # [512, 1024]
    wq_chunks = []
    for jb in range(2 * H):
        blk = qk[:, jb * P:(jb + 1) * P]       # [512, 128]
        wq_chunks.append(_pack(blk))           # [128, 512]
    wq0_p = np.ascontiguousarray(wq_chunks[0])
    wqr_p = np.ascontiguousarray(np.concatenate(wq_chunks[1:], axis=1))

    wv_p = _pack(wqkvT[:, 2 * C:3 * C])
    wp_p = _pack(wpT)
    in_maps = []
    for b in range(B):
        xT = np.ascontiguousarray(x[b].T).astype(bf)   # [512, 1024]
        xt_c = [_pack(np.ascontiguousarray(xT[:, ch * 512:(ch + 1) * 512]))
                for ch in range(NCH)]                  # each [128, 2048]
        in_maps.append({
            "wq0": wq0_p,
            "wqr": wqr_p,
            "xt0": xt_c[0],
            "xt1": xt_c[1],
            "wv": wv_p,
            "wpTp": wp_p,
            "bprow": bprow,
            "qscales": qs,
        })
    return in_maps


_NC_CACHE = {}


def get_graph():
    if "nc" not in _NC_CACHE:
        _NC_CACHE["nc"] = build_graph()
    return _NC_CACHE["nc"]


def kernel(x, w_qkv, w_proj, b_proj, w_main, w_rest, _trace=False, _trace_kwargs=None):
    nc = get_graph()
    in_maps = make_in_maps(x, w_qkv, w_proj, b_proj, w_main, w_rest)
    kw = {}
    if _trace:
        kw = {"trace": True}
        if _trace_kwargs:
            kw.update(_trace_kwargs)
    res = run_bass_kernel_spmd(nc, in_maps, core_ids=list(range(NCORES)), **kw)
    outb = np.stack([np.asarray(res.results[i]["out"], dtype=np.float32)
                     for i in range(NCORES)], axis=0)
    if _trace:
        return outb, res
    return outb
